# revision 20
# baseline (speedup 1.0000x reference)
"""Block-sparse MoE (SwiGLU, top-k of 8 experts) on 8 Trainium2 NeuronCores.

Sharding: "size-class x F-eighth" expert parallelism.
  - Routing (gate matmul + softmax + top-k, ~0.07% of FLOPs) runs on the
    host; tokens are grouped per expert.
  - Experts are sorted by token count; slot k (on EVERY core) processes
    the expert with the k-th largest count, and core i holds ffn rows
    [i*F/8, (i+1)*F/8) of every expert. Since sum(counts) = T*top_k
    exactly, every core does identical work (~513 token-equivalents vs
    556 for one-expert-per-core with these counts) and the per-core
    weight traffic stays exactly 1/8 of all expert weights (50 MB fp16).
  - Each core emits, per slot, a partial output over its F/8 rows; the
    host sums the 8 fp16 partials per expert and scatter-adds into the
    full [T, H] output.

Device kernel per core, per slot k (capacity C_k tokens, H=2048, F8=512):
  phase 1: interT[f, c] = silu(w1 @ xT) * (w3 @ xT) * wgt[c]
           (PSUM-accumulated over H; routing weight folded in here so
            phase 2 needs no per-token multiply)
  phase 2: yT[h, c]     = w2.T @ interT        (PSUM-accumulated over F8)
           PSUM -> SBUF fp16 casts alternate between the vector and
           scalar engines (gpsimd has no PSUM port).
Matmuls are fp16 (measured end-to-end rel err ~4.7e-4).
All DRAM->SBUF transfers are host-pre-tiled so every DMA is contiguous.
"""

import math
import os

import numpy as np

H = 2048          # hidden dim
F = 4096          # ffn dim per expert
E = 8             # experts
NCORES = 8
P = 128           # partitions
NH = H // P       # 16 h-tiles
F8 = F // NCORES  # 512 ffn rows per core per expert
NF8 = F8 // P     # 4 f-tiles per slot

# populated by kernel() for test harness introspection
LAST_STATS = {}

_BUILD_CACHE = {}


def _chunk_shape(count):
    """Capacity C >= count split into nch EQUAL even-width chunks <= 512."""
    c_min = max(256, count)
    n = max(1, math.ceil(c_min / 512))
    w0 = 2 * math.ceil(c_min / (2 * n))
    return n * w0, [(i * w0, w0) for i in range(n)]


def _slot_chunks(C):
    n = max(1, math.ceil(C / 512))
    w0 = C // n
    assert n * w0 == C and w0 % 2 == 0 and w0 <= 512, C
    return [(i * w0, w0) for i in range(n)]


def _build(caps, h=H):
    """Build + compile the per-core Bass program for slot capacities
    ``caps`` (tuple of token capacities, sorted descending)."""
    import concourse.bacc as bacc
    import concourse.mybir as mybir
    from concourse import tile

    AF = mybir.ActivationFunctionType
    f32 = mybir.dt.float32
    f16 = mybir.dt.float16

    nslots = len(caps)
    cmax = max(caps)
    chunks = [_slot_chunks(C) for C in caps]
    max_nch = max(len(ch) for ch in chunks)

    nc = bacc.Bacc("TRN2", target_bir_lowering=False, debug=False)

    # Host-pre-tiled DRAM layouts (every DMA below is contiguous):
    #   xt{k}  [nch, P, NH, w0]   xt[ci, p, n, c]      = x_tok[ci*w0+c, n*P+p]
    #   w13_{k}[NF8, P, 2, NH, P] w13[fi, p, m, n, j]  = w{1,3}[fi*P+j, n*P+p]
    #   w2_{k} [P, NH, NF8, P]    w2t[p, ht, fi, j]    = w2[fi*P+p, ht*P+j]
    #   wg{k}  [P, C]             broadcast routing weights
    #   yt{k}  [h, C]             fp16 partial output, yt[h, c]
    xt_d, w13_d, w2_d, wg_d, y_d = [], [], [], [], []
    for k, C in enumerate(caps):
        nch = len(chunks[k])
        w0 = chunks[k][0][1]
        xt_d.append(nc.dram_tensor(f"xt{k}", [nch, P, NH, w0], f16,
                                   kind="ExternalInput").ap())
        w13_d.append(nc.dram_tensor(f"w13_{k}", [NF8, P, 2, NH, P], f16,
                                    kind="ExternalInput").ap())
        w2_d.append(nc.dram_tensor(f"w2_{k}", [P, NH, NF8, P], f16,
                                   kind="ExternalInput").ap())
        wg_d.append(nc.dram_tensor(f"wg{k}", [P, C], f32,
                                   kind="ExternalInput").ap())
        # partition-major output so the one-DMA-per-slot store is fully
        # contiguous; the host transposes to [H, C] when combining.
        y_d.append(nc.dram_tensor(f"yt{k}", [P, NH, C], f16,
                                  kind="ExternalOutput").ap())

    with tile.TileContext(nc) as tc:
        with (
            tc.tile_pool(name="misc", bufs=1) as misc_pool,
            tc.tile_pool(name="psum", bufs=2, space="PSUM") as psum_pool,
            tc.tile_pool(name="xtp", bufs=1) as xt_pool,
            tc.tile_pool(name="wcol", bufs=4) as wcol_pool,
            tc.tile_pool(name="inter", bufs=1) as inter_pool,
            tc.tile_pool(name="wgp", bufs=1) as wg_pool,
            tc.tile_pool(name="p1tmp", bufs=2) as p1tmp,
            tc.tile_pool(name="w2col", bufs=1) as w2_pool,
            tc.tile_pool(name="obuf", bufs=1) as ob_pool,
        ):
            # PE warmup: zero-matmuls with no DMA dependencies run
            # immediately, lifting the HAM clock gate (1.2 -> 2.4 GHz)
            # while the first real loads are still in flight. Sized to
            # keep the PE busy until the first weight tiles land
            # (~12 us) so the HAM duty cycle never drops back.
            wsrc = misc_pool.tile([P, P], f16, tag="wsrc")
            nc.vector.memset(wsrc[:], 0.0)
            wps = psum_pool.tile([P, P], f32, tag="ps3", bufs=3,
                                 name="warm_ps")
            NWARM = 150
            for i in range(NWARM):
                nc.tensor.matmul(wps[:], wsrc[:], wsrc[:],
                                 start=(i == 0), stop=(i == NWARM - 1))

            # SBUF tiles reused with slot parity (slot k uses par = k%2);
            # fixed shapes so tag reuse is uniform.
            xt_tiles = [[xt_pool.tile([P, NH, 512], f16, tag=f"xt{par}{ci}",
                                      name=f"xt{par}{ci}")
                         for ci in range(max_nch)] for par in range(2)]
            it_tiles = [[inter_pool.tile([P, cmax], f16, tag=f"it{par}{fi}",
                                         name=f"it{par}{fi}")
                         for fi in range(NF8)] for par in range(2)]
            wg_tiles = [wg_pool.tile([P, cmax], f32, tag=f"wg{par}",
                                     name=f"wg{par}")
                        for par in range(2)]

            # Startup critical path: three queues in parallel.
            #   sync  (SP HWDGE, first to start ~6us): w13 slot0 col 0 in
            #         h-quarters, then the rest of the weight stream.
            #   scalar: slot0 x chunk-a in h-quarters (so hi<4 matmuls
            #         start as soon as the first quarter lands), then
            #         chunk-b (not needed until slot0's second p1 pass).
            #   gpsimd: routing weights, later the output stores.
            ch0 = chunks[0]
            w00 = ch0[0][1]
            wc0 = wcol_pool.tile([P, 2, NH, P], f16, tag="wc", name="wc0")
            hq = NH // 4
            for q in range(4):
                nc.scalar.dma_start(
                    xt_tiles[0][0][:, q * hq:(q + 1) * hq, :w00],
                    xt_d[0][0][:, q * hq:(q + 1) * hq, :])
                nc.sync.dma_start(
                    wc0[:, :, q * hq:(q + 1) * hq, :],
                    w13_d[0][0][:, :, q * hq:(q + 1) * hq, :])
            for ci in range(1, len(ch0)):
                nc.scalar.dma_start(xt_tiles[0][ci][:, :, :w00],
                                    xt_d[0][ci])
            nc.gpsimd.dma_start(wg_tiles[0][:, :caps[0]], wg_d[0][:])

            for k in range(nslots):
                par = k % 2
                C = caps[k]
                chs = chunks[k]
                nch = len(chs)
                w0 = chs[0][1]
                xts = xt_tiles[par]
                its = it_tiles[par]
                wgt = wg_tiles[par]

                # prefetch next slot's x (scalar ring) + routing weights
                # (gpsimd ring); WAR deps on slot k-1 are already clear.
                if k + 1 < nslots:
                    npar, nC = (k + 1) % 2, caps[k + 1]
                    nw0 = chunks[k + 1][0][1]
                    for ci in range(len(chunks[k + 1])):
                        nc.scalar.dma_start(
                            xt_tiles[npar][ci][:, :, :nw0],
                            xt_d[k + 1][ci])
                    nc.gpsimd.dma_start(wg_tiles[npar][:, :nC],
                                        wg_d[k + 1][:])

                # ---- phase 1: interT = silu(w1 @ xT) * (w3 @ xT) * wgt ----
                def p1_chunk(fi, wc, ci, c0, cw):
                    it = its[fi]
                    ps1 = psum_pool.tile([P, 512], f32, tag="ps1",
                                         bufs=3, name=f"ps1_{k}_{fi}_{ci}")
                    ps3 = psum_pool.tile([P, 512], f32, tag="ps3",
                                         bufs=3, name=f"ps3_{k}_{fi}_{ci}")
                    for hi in range(NH):
                        nc.tensor.matmul(
                            ps1[:, :cw], wc[:, 0, hi, :],
                            xts[ci][:, hi, :cw],
                            start=(hi == 0), stop=(hi == NH - 1))
                    for hi in range(NH):
                        nc.tensor.matmul(
                            ps3[:, :cw], wc[:, 1, hi, :],
                            xts[ci][:, hi, :cw],
                            start=(hi == 0), stop=(hi == NH - 1))
                    # silu(a)*b*w = (a*sigmoid(a)) * (b*w); sil first so
                    # it runs while the ps3 chain is still on the PE —
                    # only t2+it trail the last matmul of the group.
                    sig = p1tmp.tile([P, 512], f32, tag="sig")
                    nc.scalar.activation(sig[:, :cw], ps1[:, :cw],
                                         AF.Sigmoid)
                    sil = p1tmp.tile([P, 512], f32, tag="sil")
                    nc.vector.tensor_mul(sil[:, :cw], ps1[:, :cw],
                                         sig[:, :cw])
                    t2 = p1tmp.tile([P, 512], f32, tag="t2")
                    nc.vector.tensor_mul(t2[:, :cw], ps3[:, :cw],
                                         wgt[:, c0:c0 + cw])
                    nc.vector.tensor_mul(it[:, c0:c0 + cw],
                                         sil[:, :cw], t2[:, :cw])

                for fi in range(NF8):
                    if k == 0 and fi == 0:
                        wc = wc0          # preloaded in the startup block
                    else:
                        wc = wcol_pool.tile([P, 2, NH, P], f16, tag="wc",
                                            name=f"wc_{k}_{fi}")
                        nc.sync.dma_start(wc[:], w13_d[k][fi])
                    for ci, (c0, cw) in enumerate(chs):
                        p1_chunk(fi, wc, ci, c0, cw)

                # ---- phase 2: yT[ht, :] = w2.T @ interT ----
                # w2 is one contiguous 2 MB DMA per slot, issued after the
                # slot's w13 columns so the sync ring never idles on the
                # small per-ht tile gating. The PSUM->SBUF f16 casts
                # alternate between the vector and scalar engines and
                # accumulate into a per-slot output buffer; the slot's
                # output leaves as ONE 1.8 MB DMA on the scalar HWDGE
                # ring (per-ht stores measurably backpressure the casts
                # and stall the PE — v2/v3 traces).
                w2c = w2_pool.tile([P, NH, NF8, P], f16, tag=f"w2c{par}",
                                   name=f"w2c{par}")
                nc.sync.dma_start(w2c[:], w2_d[k])
                ob = ob_pool.tile([P, NH, cmax], f16, tag=f"ob{par}",
                                  name=f"ob{par}")
                last_slot = (k == nslots - 1)
                for ht in range(NH):
                    for ci, (c0, cw) in enumerate(chs):
                        po = psum_pool.tile([P, 512], f32, tag="po",
                                            bufs=2, name=f"po_{k}_{ht}_{ci}")
                        for fi in range(NF8):
                            nc.tensor.matmul(
                                po[:, :cw], w2c[:, ht, fi, :],
                                its[fi][:, c0:c0 + cw],
                                start=(fi == 0), stop=(fi == NF8 - 1))
                        use_dve = (ci == 0) if nch > 1 else (ht % 2 == 0)
                        if use_dve:
                            nc.vector.tensor_copy(ob[:, ht, c0:c0 + cw],
                                                  po[:, :cw])
                        else:
                            nc.scalar.copy(ob[:, ht, c0:c0 + cw],
                                           po[:, :cw])
                    if last_slot:
                        # the final slot stores per h-tile so the stores
                        # overlap its phase 2 instead of serializing a
                        # 1.8 MB DMA after the last matmul
                        nc.scalar.dma_start(y_d[k][:, ht, :C],
                                            ob[:, ht, :C])
                if not last_slot:
                    nc.scalar.dma_start(y_d[k][:], ob[:, :, :C])

    nc.compile()
    return nc


def _get_nc(caps):
    key = tuple(caps)
    if key not in _BUILD_CACHE:
        _BUILD_CACHE[key] = _build(key)
    return _BUILD_CACHE[key]


def _route(x, gate_w, top_k):
    """Host routing, matching the reference exactly:
    softmax(x @ gate_w.T) -> top-k (ties -> lower index) -> renormalize."""
    logits = x.astype(np.float64) @ gate_w.astype(np.float64).T
    m = logits.max(axis=-1, keepdims=True)
    p = np.exp(logits - m)
    p /= p.sum(axis=-1, keepdims=True)
    idx = np.argsort(-p, axis=-1, kind="stable")[:, :top_k]          # [T, k]
    vals = np.take_along_axis(p, idx, axis=-1)
    vals = vals / vals.sum(axis=-1, keepdims=True)
    return idx, vals.astype(np.float32)


def _fake_device(in_maps, caps):
    """Numpy stand-in for the device: consumes the exact tiled in_maps
    (validates host-side layouts end-to-end). Dev aid, off by default."""
    class R:
        exec_time_ns = None
        mean_exec_time_ns = None
        results = []
    res = R()
    for m in in_maps:
        out = {}
        for k, C in enumerate(caps):
            xt = m[f"xt{k}"]
            nch, _, _, w0 = xt.shape
            xs = xt.transpose(0, 3, 2, 1).reshape(C, H).astype(np.float32)
            w13 = m[f"w13_{k}"]
            w1e = w13[:, :, 0].transpose(0, 3, 2, 1).reshape(F8, H).astype(
                np.float32)
            w3e = w13[:, :, 1].transpose(0, 3, 2, 1).reshape(F8, H).astype(
                np.float32)
            w2e = m[f"w2_{k}"].transpose(2, 0, 1, 3).reshape(F8, H).astype(
                np.float32)
            wgt = m[f"wg{k}"][0]
            h1 = xs @ w1e.T
            h3 = xs @ w3e.T
            inter = (h1 / (1 + np.exp(-h1))) * h3 * wgt[:, None]
            y = inter @ w2e                                   # [C, H]
            out[f"yt{k}"] = np.ascontiguousarray(
                y.T.reshape(NH, P, C).transpose(1, 0, 2)).astype(np.float16)
        res.results.append(out)
    return res


def kernel(x, gate_w, w1, w2, w3, top_k):
    from concourse.bass_utils import run_bass_kernel_spmd

    x = np.ascontiguousarray(np.asarray(x, dtype=np.float32))
    gate_w = np.asarray(gate_w, dtype=np.float32)
    w1 = np.asarray(w1, dtype=np.float32)
    w2 = np.asarray(w2, dtype=np.float32)
    w3 = np.asarray(w3, dtype=np.float32)
    k = int(np.asarray(top_k))
    t, h = x.shape
    e = gate_w.shape[0]
    f = w1.shape[0] // e
    assert (h, f, e) == (H, F, E), (h, f, e)

    idx, vals = _route(x, gate_w, k)                                  # [T, k]

    # token lists per expert
    tok_lists = []
    wgt_lists = []
    for ei in range(E):
        tok_i, slot_i = np.nonzero(idx == ei)
        tok_lists.append(tok_i.astype(np.int64))
        wgt_lists.append(vals[tok_i, slot_i].astype(np.float32))
    counts = np.array([len(ti) for ti in tok_lists])
    sigma = np.argsort(-counts, kind="stable")       # experts by size desc
    caps = tuple(_chunk_shape(counts[sigma[kk]])[0] for kk in range(E))

    xmm = x.astype(np.float16)
    # per-slot tensors shared by every core (same expert, same tokens)
    shared = []
    for kk in range(E):
        ei = int(sigma[kk])
        tok = tok_lists[ei]
        n = len(tok)
        C = caps[kk]
        chs = _slot_chunks(C)
        nch, w0 = len(chs), chs[0][1]
        xs = np.zeros((C, H), dtype=np.float16)
        xs[:n] = xmm[tok]
        xt = np.ascontiguousarray(
            xs.reshape(nch, w0, NH, P).transpose(0, 3, 2, 1))
        wg = np.zeros(C, dtype=np.float32)
        wg[:n] = wgt_lists[ei]
        wgb = np.ascontiguousarray(np.broadcast_to(wg, (P, C)))
        shared.append((xt, wgb))

    in_maps = []
    for ii in range(NCORES):
        im = {}
        for kk in range(E):
            ei = int(sigma[kk])
            r0 = ei * F + ii * F8
            w1s = w1[r0:r0 + F8].astype(np.float16)
            w3s = w3[r0:r0 + F8].astype(np.float16)
            w2s = w2[r0:r0 + F8].astype(np.float16)
            w13t = np.ascontiguousarray(np.stack(
                [w1s.reshape(NF8, P, NH, P).transpose(0, 3, 2, 1),
                 w3s.reshape(NF8, P, NH, P).transpose(0, 3, 2, 1)], axis=2))
            w2t = np.ascontiguousarray(
                w2s.reshape(NF8, P, NH, P).transpose(1, 2, 0, 3))
            im[f"xt{kk}"] = shared[kk][0]
            im[f"wg{kk}"] = shared[kk][1]
            im[f"w13_{kk}"] = w13t
            im[f"w2_{kk}"] = w2t
        in_maps.append(im)

    if os.environ.get("MOE_FAKE"):
        res = _fake_device(in_maps, caps)
    else:
        nc = _get_nc(caps)
        trace = bool(int(os.environ.get("MOE_TRACE", "0")))
        res = run_bass_kernel_spmd(nc, in_maps, core_ids=list(range(NCORES)),
                                   trace=trace)
    LAST_STATS.clear()
    LAST_STATS.update({
        "caps": caps,
        "exec_time_ns": res.exec_time_ns,
        "mean_exec_time_ns": res.mean_exec_time_ns,
        "counts": counts.tolist(),
    })

    out = np.zeros((t, h), dtype=np.float32)
    for kk in range(E):
        ei = int(sigma[kk])
        n = len(tok_lists[ei])
        acc = np.zeros(res.results[0][f"yt{kk}"].shape, dtype=np.float32)
        for ii in range(NCORES):
            acc += res.results[ii][f"yt{kk}"]                 # [P, NH, C]
        acc = acc.transpose(1, 0, 2).reshape(h, -1)           # [H, C]
        out[tok_lists[ei]] += acc[:, :n].T
    return out


# revision 26
# speedup vs baseline: 1.0938x; 1.0938x over previous
"""Block-sparse MoE (SwiGLU, top-k of 8 experts) on 8 Trainium2 NeuronCores.

Sharding: "size-class x F-eighth" expert parallelism.
  - Routing (gate matmul + softmax + top-k, ~0.07% of FLOPs) runs on the
    host; tokens are grouped per expert.
  - Experts are sorted by token count; slot k (on EVERY core) processes
    the expert with the k-th largest count, and core i holds ffn rows
    [i*F/8, (i+1)*F/8) of every expert. Since sum(counts) = T*top_k
    exactly, every core does identical work (~513 token-equivalents vs
    556 for one-expert-per-core with these counts) and the per-core
    weight traffic stays exactly 1/8 of all expert weights (50 MB fp16).
  - Each core emits, per slot, a partial output over its F/8 rows; the
    host sums the 8 fp16 partials per expert and scatter-adds into the
    full [T, H] output.

Device kernel per core, per slot k (capacity C_k tokens, H=2048, F8=512):
  phase 1: interT[f, c] = silu(w1 @ xT) * (w3 @ xT) * wgt[c]
           (PSUM-accumulated over H; routing weight folded in here so
            phase 2 needs no per-token multiply)
  phase 2: yT[h, c]     = w2.T @ interT        (PSUM-accumulated over F8)
           PSUM -> SBUF fp16 casts alternate between the vector and
           scalar engines (gpsimd has no PSUM port).
Matmuls are fp16 (measured end-to-end rel err ~4.7e-4).
All DRAM->SBUF transfers are host-pre-tiled so every DMA is contiguous.
"""

import math
import os

import numpy as np

H = 2048          # hidden dim
F = 4096          # ffn dim per expert
E = 8             # experts
NCORES = 8
P = 128           # partitions
NH = H // P       # 16 h-tiles
F8 = F // NCORES  # 512 ffn rows per core per expert
NF8 = F8 // P     # 4 f-tiles per slot

# populated by kernel() for test harness introspection
LAST_STATS = {}

_BUILD_CACHE = {}


def _chunk_shape(count):
    """Capacity C >= count split into nch EQUAL even-width chunks <= 512."""
    c_min = max(256, count)
    n = max(1, math.ceil(c_min / 512))
    w0 = 2 * math.ceil(c_min / (2 * n))
    return n * w0, [(i * w0, w0) for i in range(n)]


def _slot_chunks(C):
    n = max(1, math.ceil(C / 512))
    w0 = C // n
    assert n * w0 == C and w0 % 2 == 0 and w0 <= 512, C
    return [(i * w0, w0) for i in range(n)]


def _build(caps, h=H):
    """Build + compile the per-core Bass program for slot capacities
    ``caps`` (tuple of token capacities, sorted descending)."""
    import concourse.bacc as bacc
    import concourse.mybir as mybir
    from concourse import tile

    AF = mybir.ActivationFunctionType
    f32 = mybir.dt.float32
    f16 = mybir.dt.float16

    nslots = len(caps)
    cmax = max(caps)
    chunks = [_slot_chunks(C) for C in caps]
    max_nch = max(len(ch) for ch in chunks)

    nc = bacc.Bacc("TRN2", target_bir_lowering=False, debug=False)

    # Host-pre-tiled DRAM layouts (every DMA below is contiguous):
    #   xt{k}  [nch, P, NH, w0]   xt[ci, p, n, c]      = x_tok[ci*w0+c, n*P+p]
    #   w13_{k}[NF8, P, 2, NH, P] w13[fi, p, m, n, j]  = w{1,3}[fi*P+j, n*P+p]
    #   w2_{k} [P, NH, NF8, P]    w2t[p, ht, fi, j]    = w2[fi*P+p, ht*P+j]
    #   wg{k}  [P, C]             broadcast routing weights
    #   yt{k}  [h, C]             fp16 partial output, yt[h, c]
    xt_d, w13_d, w2_d, wg_d, y_d = [], [], [], [], []
    for k, C in enumerate(caps):
        nch = len(chunks[k])
        w0 = chunks[k][0][1]
        xt_d.append(nc.dram_tensor(f"xt{k}", [nch, P, NH, w0], f16,
                                   kind="ExternalInput").ap())
        w13_d.append(nc.dram_tensor(f"w13_{k}", [NF8, P, 2, NH, P], f16,
                                    kind="ExternalInput").ap())
        w2_d.append(nc.dram_tensor(f"w2_{k}", [P, NH, NF8, P], f16,
                                   kind="ExternalInput").ap())
        wg_d.append(nc.dram_tensor(f"wg{k}", [P, C], f32,
                                   kind="ExternalInput").ap())
        # partition-major output so the one-DMA-per-slot store is fully
        # contiguous; the host transposes to [H, C] when combining.
        y_d.append(nc.dram_tensor(f"yt{k}", [P, NH, C], f16,
                                  kind="ExternalOutput").ap())

    with tile.TileContext(nc) as tc:
        with (
            tc.tile_pool(name="misc", bufs=1) as misc_pool,
            tc.tile_pool(name="psum", bufs=2, space="PSUM") as psum_pool,
            tc.tile_pool(name="xtp", bufs=1) as xt_pool,
            tc.tile_pool(name="wcol", bufs=4) as wcol_pool,
            tc.tile_pool(name="inter", bufs=1) as inter_pool,
            tc.tile_pool(name="wgp", bufs=1) as wg_pool,
            tc.tile_pool(name="p1tmp", bufs=2) as p1tmp,
            tc.tile_pool(name="w2col", bufs=1) as w2_pool,
            tc.tile_pool(name="obuf", bufs=1) as ob_pool,
        ):
            # PE warmup: zero-matmuls with no DMA dependencies run
            # immediately, lifting the HAM clock gate (1.2 -> 2.4 GHz)
            # while the first real loads are still in flight. Sized to
            # keep the PE busy until the first weight tiles land
            # (~12 us) so the HAM duty cycle never drops back.
            wsrc = misc_pool.tile([P, P], f16, tag="wsrc")
            nc.vector.memset(wsrc[:], 0.0)
            wps = psum_pool.tile([P, P], f32, tag="ps1", bufs=3,
                                 name="warm_ps")
            NWARM = 150
            for i in range(NWARM):
                nc.tensor.matmul(wps[:], wsrc[:], wsrc[:],
                                 start=(i == 0), stop=(i == NWARM - 1))

            # SBUF tiles reused with slot parity (slot k uses par = k%2);
            # fixed shapes so tag reuse is uniform.
            xt_tiles = [[xt_pool.tile([P, NH, 512], f16, tag=f"xt{par}{ci}",
                                      name=f"xt{par}{ci}")
                         for ci in range(max_nch)] for par in range(2)]
            it_tiles = [[inter_pool.tile([P, cmax], f16, tag=f"it{par}{fi}",
                                         name=f"it{par}{fi}")
                         for fi in range(NF8)] for par in range(2)]
            wg_tiles = [wg_pool.tile([P, cmax], f32, tag=f"wg{par}",
                                     name=f"wg{par}")
                        for par in range(2)]

            # Startup critical path: three queues in parallel.
            #   sync  (SP HWDGE, first to start ~6us): w13 slot0 col 0 in
            #         h-quarters, then the rest of the weight stream.
            #   scalar: slot0 x chunk-a in h-quarters (so hi<4 matmuls
            #         start as soon as the first quarter lands), then
            #         chunk-b (not needed until slot0's second p1 pass).
            #   gpsimd: routing weights, later the output stores.
            # Early descriptor ISSUE is the startup bottleneck (engines
            # fetch instructions at degraded rates for the first ~10us),
            # so keep the count low: h-halves for the first column/x,
            # whole-chunk DMAs after.
            ch0 = chunks[0]
            w00 = ch0[0][1]
            wc0 = wcol_pool.tile([P, 2, NH, P], f16, tag="wc", name="wc0")
            h2 = NH // 2
            for q in range(2):
                nc.scalar.dma_start(
                    xt_tiles[0][0][:, q * h2:(q + 1) * h2, :w00],
                    xt_d[0][0][:, q * h2:(q + 1) * h2, :])
                nc.sync.dma_start(
                    wc0[:, :, q * h2:(q + 1) * h2, :],
                    w13_d[0][0][:, :, q * h2:(q + 1) * h2, :])
            for ci in range(1, len(ch0)):
                nc.scalar.dma_start(xt_tiles[0][ci][:, :, :w00],
                                    xt_d[0][ci])
            nc.gpsimd.dma_start(wg_tiles[0][:, :caps[0]], wg_d[0][:])

            for k in range(nslots):
                par = k % 2
                C = caps[k]
                chs = chunks[k]
                nch = len(chs)
                w0 = chs[0][1]
                xts = xt_tiles[par]
                its = it_tiles[par]
                wgt = wg_tiles[par]

                # prefetch next slot's x (scalar ring) + routing weights
                # (gpsimd ring); WAR deps on slot k-1 are already clear.
                if k + 1 < nslots:
                    npar, nC = (k + 1) % 2, caps[k + 1]
                    nw0 = chunks[k + 1][0][1]
                    for ci in range(len(chunks[k + 1])):
                        nc.scalar.dma_start(
                            xt_tiles[npar][ci][:, :, :nw0],
                            xt_d[k + 1][ci])
                    nc.gpsimd.dma_start(wg_tiles[npar][:, :nC],
                                        wg_d[k + 1][:])

                # ---- phase 1: interT = silu(w1 @ xT) * (w3 @ xT) * wgt ----
                def p1_chunk(fi, wc, ci, c0, cw):
                    it = its[fi]
                    ps1 = psum_pool.tile([P, 512], f32, tag="ps1",
                                         bufs=3, name=f"ps1_{k}_{fi}_{ci}")
                    ps3 = psum_pool.tile([P, 512], f32, tag="ps3",
                                         bufs=2, name=f"ps3_{k}_{fi}_{ci}")
                    for hi in range(NH):
                        nc.tensor.matmul(
                            ps1[:, :cw], wc[:, 0, hi, :],
                            xts[ci][:, hi, :cw],
                            start=(hi == 0), stop=(hi == NH - 1))
                    for hi in range(NH):
                        nc.tensor.matmul(
                            ps3[:, :cw], wc[:, 1, hi, :],
                            xts[ci][:, hi, :cw],
                            start=(hi == 0), stop=(hi == NH - 1))
                    # silu(a)*b = (a*sigmoid(a)) * b; the routing weight
                    # is folded into the phase-2 PSUM drain instead (one
                    # fewer full pass over inter -> less DVE energy; the
                    # run is power-throttle sensitive).
                    sig = p1tmp.tile([P, 512], f32, tag="sig")
                    nc.scalar.activation(sig[:, :cw], ps1[:, :cw],
                                         AF.Sigmoid)
                    sil = p1tmp.tile([P, 512], f32, tag="sil")
                    nc.vector.tensor_mul(sil[:, :cw], ps1[:, :cw],
                                         sig[:, :cw])
                    nc.vector.tensor_mul(it[:, c0:c0 + cw],
                                         sil[:, :cw], ps3[:, :cw])

                for fi in range(NF8):
                    if k == 0 and fi == 0:
                        wc = wc0          # preloaded in the startup block
                    else:
                        wc = wcol_pool.tile([P, 2, NH, P], f16, tag="wc",
                                            name=f"wc_{k}_{fi}")
                        nc.sync.dma_start(wc[:], w13_d[k][fi])
                    for ci, (c0, cw) in enumerate(chs):
                        p1_chunk(fi, wc, ci, c0, cw)

                # ---- phase 2: yT[ht, :] = w2.T @ interT ----
                # w2 is one contiguous 2 MB DMA per slot, issued after the
                # slot's w13 columns so the sync ring never idles on the
                # small per-ht tile gating. The PSUM->SBUF f16 casts
                # alternate between the vector and scalar engines and
                # accumulate into a per-slot output buffer; the slot's
                # output leaves as ONE 1.8 MB DMA on the scalar HWDGE
                # ring (per-ht stores measurably backpressure the casts
                # and stall the PE — v2/v3 traces).
                w2c = w2_pool.tile([P, NH, NF8, P], f16, tag=f"w2c{par}",
                                   name=f"w2c{par}")
                nc.sync.dma_start(w2c[:], w2_d[k])
                ob = ob_pool.tile([P, NH, cmax], f16, tag=f"ob{par}",
                                  name=f"ob{par}")
                last_slot = (k == nslots - 1)
                for ht in range(NH):
                    for ci, (c0, cw) in enumerate(chs):
                        po = psum_pool.tile([P, 512], f32, tag="po",
                                            bufs=3, name=f"po_{k}_{ht}_{ci}")
                        for fi in range(NF8):
                            nc.tensor.matmul(
                                po[:, :cw], w2c[:, ht, fi, :],
                                its[fi][:, c0:c0 + cw],
                                start=(fi == 0), stop=(fi == NF8 - 1))
                        # drain PSUM with the routing weight applied
                        nc.vector.tensor_mul(ob[:, ht, c0:c0 + cw],
                                             po[:, :cw],
                                             wgt[:, c0:c0 + cw])
                    if last_slot:
                        # the final slot stores per h-tile so the stores
                        # overlap its phase 2 instead of serializing a
                        # 1.8 MB DMA after the last matmul
                        nc.scalar.dma_start(y_d[k][:, ht, :C],
                                            ob[:, ht, :C])
                if not last_slot:
                    nc.scalar.dma_start(y_d[k][:], ob[:, :, :C])

    nc.compile()
    return nc


def _get_nc(caps):
    key = tuple(caps)
    if key not in _BUILD_CACHE:
        _BUILD_CACHE[key] = _build(key)
    return _BUILD_CACHE[key]


def _route(x, gate_w, top_k):
    """Host routing, matching the reference exactly:
    softmax(x @ gate_w.T) -> top-k (ties -> lower index) -> renormalize."""
    logits = x.astype(np.float64) @ gate_w.astype(np.float64).T
    m = logits.max(axis=-1, keepdims=True)
    p = np.exp(logits - m)
    p /= p.sum(axis=-1, keepdims=True)
    idx = np.argsort(-p, axis=-1, kind="stable")[:, :top_k]          # [T, k]
    vals = np.take_along_axis(p, idx, axis=-1)
    vals = vals / vals.sum(axis=-1, keepdims=True)
    return idx, vals.astype(np.float32)


def _fake_device(in_maps, caps):
    """Numpy stand-in for the device: consumes the exact tiled in_maps
    (validates host-side layouts end-to-end). Dev aid, off by default."""
    class R:
        exec_time_ns = None
        mean_exec_time_ns = None
        results = []
    res = R()
    for m in in_maps:
        out = {}
        for k, C in enumerate(caps):
            xt = m[f"xt{k}"]
            nch, _, _, w0 = xt.shape
            xs = xt.transpose(0, 3, 2, 1).reshape(C, H).astype(np.float32)
            w13 = m[f"w13_{k}"]
            w1e = w13[:, :, 0].transpose(0, 3, 2, 1).reshape(F8, H).astype(
                np.float32)
            w3e = w13[:, :, 1].transpose(0, 3, 2, 1).reshape(F8, H).astype(
                np.float32)
            w2e = m[f"w2_{k}"].transpose(2, 0, 1, 3).reshape(F8, H).astype(
                np.float32)
            wgt = m[f"wg{k}"][0]
            h1 = xs @ w1e.T
            h3 = xs @ w3e.T
            inter = (h1 / (1 + np.exp(-h1))) * h3
            y = (inter @ w2e) * wgt[:, None]                  # [C, H]
            out[f"yt{k}"] = np.ascontiguousarray(
                y.T.reshape(NH, P, C).transpose(1, 0, 2)).astype(np.float16)
        res.results.append(out)
    return res


def kernel(x, gate_w, w1, w2, w3, top_k):
    from concourse.bass_utils import run_bass_kernel_spmd

    x = np.ascontiguousarray(np.asarray(x, dtype=np.float32))
    gate_w = np.asarray(gate_w, dtype=np.float32)
    w1 = np.asarray(w1, dtype=np.float32)
    w2 = np.asarray(w2, dtype=np.float32)
    w3 = np.asarray(w3, dtype=np.float32)
    k = int(np.asarray(top_k))
    t, h = x.shape
    e = gate_w.shape[0]
    f = w1.shape[0] // e
    assert (h, f, e) == (H, F, E), (h, f, e)

    idx, vals = _route(x, gate_w, k)                                  # [T, k]

    # token lists per expert
    tok_lists = []
    wgt_lists = []
    for ei in range(E):
        tok_i, slot_i = np.nonzero(idx == ei)
        tok_lists.append(tok_i.astype(np.int64))
        wgt_lists.append(vals[tok_i, slot_i].astype(np.float32))
    counts = np.array([len(ti) for ti in tok_lists])
    sigma = np.argsort(-counts, kind="stable")       # experts by size desc
    caps = tuple(_chunk_shape(counts[sigma[kk]])[0] for kk in range(E))

    xmm = x.astype(np.float16)
    # per-slot tensors shared by every core (same expert, same tokens)
    shared = []
    for kk in range(E):
        ei = int(sigma[kk])
        tok = tok_lists[ei]
        n = len(tok)
        C = caps[kk]
        chs = _slot_chunks(C)
        nch, w0 = len(chs), chs[0][1]
        xs = np.zeros((C, H), dtype=np.float16)
        xs[:n] = xmm[tok]
        xt = np.ascontiguousarray(
            xs.reshape(nch, w0, NH, P).transpose(0, 3, 2, 1))
        wg = np.zeros(C, dtype=np.float32)
        wg[:n] = wgt_lists[ei]
        wgb = np.ascontiguousarray(np.broadcast_to(wg, (P, C)))
        shared.append((xt, wgb))

    in_maps = []
    for ii in range(NCORES):
        im = {}
        for kk in range(E):
            ei = int(sigma[kk])
            r0 = ei * F + ii * F8
            w1s = w1[r0:r0 + F8].astype(np.float16)
            w3s = w3[r0:r0 + F8].astype(np.float16)
            w2s = w2[r0:r0 + F8].astype(np.float16)
            w13t = np.ascontiguousarray(np.stack(
                [w1s.reshape(NF8, P, NH, P).transpose(0, 3, 2, 1),
                 w3s.reshape(NF8, P, NH, P).transpose(0, 3, 2, 1)], axis=2))
            w2t = np.ascontiguousarray(
                w2s.reshape(NF8, P, NH, P).transpose(1, 2, 0, 3))
            im[f"xt{kk}"] = shared[kk][0]
            im[f"wg{kk}"] = shared[kk][1]
            im[f"w13_{kk}"] = w13t
            im[f"w2_{kk}"] = w2t
        in_maps.append(im)

    if os.environ.get("MOE_FAKE"):
        res = _fake_device(in_maps, caps)
    else:
        nc = _get_nc(caps)
        trace = bool(int(os.environ.get("MOE_TRACE", "0")))
        res = run_bass_kernel_spmd(nc, in_maps, core_ids=list(range(NCORES)),
                                   trace=trace)
    LAST_STATS.clear()
    LAST_STATS.update({
        "caps": caps,
        "exec_time_ns": res.exec_time_ns,
        "mean_exec_time_ns": res.mean_exec_time_ns,
        "counts": counts.tolist(),
    })

    out = np.zeros((t, h), dtype=np.float32)
    for kk in range(E):
        ei = int(sigma[kk])
        n = len(tok_lists[ei])
        acc = np.zeros(res.results[0][f"yt{kk}"].shape, dtype=np.float32)
        for ii in range(NCORES):
            acc += res.results[ii][f"yt{kk}"]                 # [P, NH, C]
        acc = acc.transpose(1, 0, 2).reshape(h, -1)           # [H, C]
        out[tok_lists[ei]] += acc[:, :n].T
    return out
